# revision 1
# baseline (speedup 1.0000x reference)
"""MiMoV2 sparse attention (GQA + sliding window + sink) on 8 TRN2 cores.

Tensor-parallel over heads: core c owns q heads 4c..4c+3 and kv head c
(GQA groups align with cores), wq/wk/wv output-dim and wo input-dim
sharded, partial o_proj outputs summed on the host.

Per-core dataflow (all feature-major / transposed layouts):
  A) hT = transpose(h) tiles via PE; qT/kT/vT = w.T @ hT (fp32r, N=512);
     RoPE applied to qT/kT in [d, tok] layout; v transposed to [tok, d].
  B) per (q-tile 512, head): S^T[k,q] = kT.T @ qT; w = exp(S^T) (bf16);
     partial-visibility tiles multiplied by precomputed 0/1 masks;
     attnT += v.T @ w (accumulating); denom += ones.T @ w ([1,q]);
     attnT *= broadcast(1/(denom + exp(sink))).
  C) out[q, :] += attnT.T @ wo  (accumulate 4 heads), DMA partial out.

Softmax uses a constant (zero) max-shift: scores for this problem are
bounded far below fp32 exp overflow, and softmax is shift-invariant, so
the result is exact; the sink logit enters the denominator as exp(sink).
"""
import os
import numpy as np
import ml_dtypes

import concourse.bass as bass
import concourse.mybir as mybir
import concourse.tile as tile
from concourse import bacc
from concourse.bass_utils import run_bass_kernel_spmd
from concourse.masks import make_identity
from contextlib import ExitStack

F32 = mybir.dt.float32
F32R = mybir.dt.float32r
BF16 = mybir.dt.bfloat16

S = 2048
HID = 4096
NQ = 32
NKV = 8
D = 128
WINDOW = 1024
THETA = 1e6
CORES = 8
QH = NQ // CORES          # 4 q heads per core
DQ = QH * D               # 512
NT = S // 512             # 4 token tiles of 512
KS = S // 128             # 16 key subtiles of 128

last_results = None       # set by kernel(); test.py reads exec_time_ns


def _schedule(positions):
    """Static attention schedule from the actual positions array.

    Returns (masks_np [128, P*512] bf16, sched[qt] = list of (ks, pidx))
    where pidx is None for fully-visible key subtiles.
    """
    pos = np.asarray(positions).astype(np.int64)
    vis = (pos[None, :] <= pos[:, None]) & (pos[:, None] - pos[None, :] < WINDOW)
    patterns = {}
    plist = []
    sched = []
    for qt in range(NT):
        row = []
        for ks in range(KS):
            sub = vis[qt * 512:(qt + 1) * 512, ks * 128:(ks + 1) * 128]
            if not sub.any():
                continue
            if sub.all():
                row.append((ks, None))
            else:
                pat = np.ascontiguousarray(sub.T).astype(np.float32)  # [128 k, 512 q]
                key = pat.tobytes()
                if key not in patterns:
                    patterns[key] = len(plist)
                    plist.append(pat)
                row.append((ks, patterns[key]))
        sched.append(row)
    if not plist:
        plist = [np.ones((128, 512), np.float32)]
    masks = np.concatenate(plist, axis=1).astype(ml_dtypes.bfloat16)  # [128, P*512]
    return masks, sched, len(plist)


def _build(sched, n_patterns):
    nc = bacc.Bacc("TRN2", target_bir_lowering=False)

    HT = nc.dram_tensor("ht", [HID, S], F32R, kind="ExternalInput")
    Wq = nc.dram_tensor("wq", [HID, DQ], F32R, kind="ExternalInput")
    Wk = nc.dram_tensor("wk", [HID, D], F32R, kind="ExternalInput")
    Wv = nc.dram_tensor("wv", [HID, D], F32R, kind="ExternalInput")
    Wo = nc.dram_tensor("wo", [DQ, HID], F32R, kind="ExternalInput")
    Cos = nc.dram_tensor("cos", [128, S], F32, kind="ExternalInput")
    Sin = nc.dram_tensor("sin", [128, S], F32, kind="ExternalInput")
    Mk = nc.dram_tensor("mk", [128, n_patterns * 512], BF16, kind="ExternalInput")
    One = nc.dram_tensor("one", [128, 1], BF16, kind="ExternalInput")
    Esk = nc.dram_tensor("esk", [1, QH], F32, kind="ExternalInput")
    Out = nc.dram_tensor("out", [S, HID], F32, kind="ExternalOutput")

    with tile.TileContext(nc) as tc, ExitStack() as top:
        persist = top.enter_context(tc.tile_pool(name="persist", bufs=1))
        ident_bf = persist.tile([128, 128], BF16)
        make_identity(nc, ident_bf[:])
        ones = persist.tile([128, 1], BF16)
        nc.sync.dma_start(ones[:], One[:])
        esk = persist.tile([1, QH], F32)
        nc.sync.dma_start(esk[:], Esk[:])
        mk_sb = persist.tile([128, n_patterns * 512], BF16)
        # persistent activations
        qT = [[persist.tile([128, 512], F32R, tag=f"qT{m}_{n}", name=f"qT{m}_{n}")
               for n in range(NT)] for m in range(QH)]
        kT = [persist.tile([128, 512], F32R, tag=f"kT{n}", name=f"kT{n}") for n in range(NT)]
        v_sb = [persist.tile([128, 512], BF16, tag=f"v{n}", name=f"v{n}") for n in range(NT)]

        ps_tr = top.enter_context(tc.tile_pool(name="ps_tr", bufs=1, space="PSUM"))
        pend_v = []

        def emit_vtr():
            while pend_v:
                n, vt = pend_v.pop(0)
                trv = ps_tr.tile([128, 512], BF16, tag="tr", name="trv")
                for t in range(4):
                    nc.tensor.transpose(trv[:, t * 128:(t + 1) * 128],
                                        vt[:, t * 128:(t + 1) * 128], ident_bf[:])
                nc.vector.tensor_copy(v_sb[n][:], trv[:])

        # ---------------- Phase A: projections + RoPE -----------------
        with ExitStack() as pa:
            wq_sb = pa.enter_context(tc.tile_pool(name="wq", bufs=1)).tile([128, 32 * DQ], F32R)
            wk_sb = pa.enter_context(tc.tile_pool(name="wk", bufs=1)).tile([128, 32 * D], F32R)
            wv_sb = pa.enter_context(tc.tile_pool(name="wv", bufs=1)).tile([128, 32 * D], F32R)

            def load_w_quarter(q):
                # weights for kt in [8q, 8q+8): wq chunks 2q, 2q+1; wk/wv quarter q
                for c8 in (2 * q, 2 * q + 1):
                    nc.sync.dma_start(
                        wq_sb[:, c8 * 4 * DQ:(c8 + 1) * 4 * DQ].rearrange("p (kt m) -> p kt m", kt=4),
                        Wq[c8 * 512:(c8 + 1) * 512, :].rearrange("(kt p) m -> p kt m", p=128))
                nc.sync.dma_start(
                    wk_sb[:, q * 8 * D:(q + 1) * 8 * D].rearrange("p (kt m) -> p kt m", kt=8),
                    Wk[q * 1024:(q + 1) * 1024, :].rearrange("(kt p) m -> p kt m", p=128))
                nc.sync.dma_start(
                    wv_sb[:, q * 8 * D:(q + 1) * 8 * D].rearrange("p (kt m) -> p kt m", kt=8),
                    Wv[q * 1024:(q + 1) * 1024, :].rearrange("(kt p) m -> p kt m", p=128))

            hTq = pa.enter_context(tc.tile_pool(name="hTq", bufs=2))
            cs = pa.enter_context(tc.tile_pool(name="cs", bufs=2))
            rtmp = pa.enter_context(tc.tile_pool(name="rtmp", bufs=2))
            vtmp = pa.enter_context(tc.tile_pool(name="vtmp", bufs=2))
            ps_proj = pa.enter_context(tc.tile_pool(name="ps_proj", bufs=1, space="PSUM"))

            def lhsT_w(m, kt):
                if m < QH:
                    return wq_sb[:, kt * DQ + m * 128:kt * DQ + (m + 1) * 128]
                if m == QH:
                    return wk_sb[:, kt * D:(kt + 1) * D]
                return wv_sb[:, kt * D:(kt + 1) * D]

            for n in range(NT):
                ps_m = [ps_proj.tile([128, 512], F32, tag=f"proj{m}", name=f"proj{m}") for m in range(QH + 2)]
                for quarter in range(4):
                    if n == 0:
                        load_w_quarter(quarter)
                    hq = hTq.tile([128, 8 * 512], F32R, tag="hq")
                    nc.sync.dma_start(
                        hq[:].rearrange("p (kt tok) -> p kt tok", kt=8),
                        HT[quarter * 1024:(quarter + 1) * 1024,
                           n * 512:(n + 1) * 512].rearrange("(kt p) tok -> p kt tok", p=128))
                    for k in range(8):
                        kt = quarter * 8 + k
                        for m in range(QH + 2):
                            nc.tensor.matmul(ps_m[m][:], lhsT_w(m, kt),
                                             hq[:, k * 512:(k + 1) * 512],
                                             start=(kt == 0), stop=(kt == 31))
                # RoPE on q heads + k; v transpose (deferred for last n-tile)
                co = cs.tile([128, 512], F32, tag="co")
                nc.sync.dma_start(co[:], Cos[:, n * 512:(n + 1) * 512])
                si = cs.tile([128, 512], F32, tag="si")
                nc.sync.dma_start(si[:], Sin[:, n * 512:(n + 1) * 512])

                # psum-touching ops first (frees proj banks asap), then
                # the SBUF-only finish ops; DVE sem ticks let PE proceed
                # as soon as the bank-freeing reads complete.
                for m in range(QH + 1):
                    dst = qT[m][n][:] if m < QH else kT[n][:]
                    pst = rtmp.tile([128, 512], F32, tag="pst", name="pst")
                    nc.scalar.copy(pst[:], ps_m[m][:])
                    t2 = rtmp.tile([128, 512], F32, tag="t2", name="t2")
                    nc.vector.tensor_mul(t2[0:64, :], ps_m[m][64:128, :], si[0:64, :])
                    nc.vector.tensor_mul(t2[64:128, :], ps_m[m][0:64, :], si[64:128, :])
                    nc.vector.tensor_mul(dst, pst[:], co[:])
                    nc.vector.tensor_add(dst, dst, t2[:])
                vt = vtmp.tile([128, 512], BF16, tag="vt", name=f"vt{n}")
                nc.scalar.copy(vt[:], ps_m[QH + 1][:])
                pend_v.append((n, vt))
                if n < NT - 1:
                    emit_vtr()
                if n < NT - 1:
                    emit_vtr()

        # ------------- Phase B+C: attention + o_proj ------------------
        with ExitStack() as pb:
            nc.sync.dma_start(mk_sb[:], Mk[:])
            wopool = pb.enter_context(tc.tile_pool(name="wopool", bufs=3))
            wpool = pb.enter_context(tc.tile_pool(name="wpool", bufs=8))
            apool = pb.enter_context(tc.tile_pool(name="apool", bufs=5))
            dpool = pb.enter_context(tc.tile_pool(name="dpool", bufs=2))
            opool = pb.enter_context(tc.tile_pool(name="opool", bufs=5))
            ps_s = pb.enter_context(tc.tile_pool(name="ps_s", bufs=2, space="PSUM"))
            ps_a = pb.enter_context(tc.tile_pool(name="ps_a", bufs=2, space="PSUM"))
            ps_d = pb.enter_context(tc.tile_pool(name="ps_d", bufs=1, space="PSUM"))
            ps_o = pb.enter_context(tc.tile_pool(name="ps_o", bufs=2, space="PSUM"))

            def oproj_chunk(qt, attnT, oc):
                woc = wopool.tile([128, QH * 512], F32R, tag="woc", name="woc")
                nc.sync.dma_start(
                    woc[:].rearrange("p (dt m) -> p dt m", dt=QH),
                    Wo[:, oc * 512:(oc + 1) * 512].rearrange("(dt p) m -> p dt m", p=128))
                for qs in range(4):
                    o_ps = ps_o.tile([128, 512], F32, tag="o", name="o_ps")
                    for hd in range(QH):
                        nc.tensor.matmul(
                            o_ps[:],
                            attnT[hd][:, qs * 128:(qs + 1) * 128],
                            woc[:, hd * 512:(hd + 1) * 512],
                            start=(hd == 0), stop=(hd == QH - 1))
                    ob = opool.tile([128, 512], F32, tag="ob", name="ob")
                    if qs % 2 == 0:
                        nc.vector.tensor_copy(ob[:], o_ps[:])
                    else:
                        nc.scalar.copy(ob[:], o_ps[:])
                    nc.gpsimd.dma_start(
                        Out[qt * 512 + qs * 128:qt * 512 + (qs + 1) * 128,
                            oc * 512:(oc + 1) * 512], ob[:])

            prev = None  # (qt, attnT) pending o_proj
            for qt in range(NT):
                if qt == NT - 1:
                    emit_vtr()
                row = sched[qt]
                attnT = []
                for hd in range(QH):
                    if prev is not None:
                        for oc in range(2 * hd, 2 * hd + 2):
                            oproj_chunk(prev[0], prev[1], oc)
                    a_ps = ps_a.tile([128, 512], F32, tag="a")
                    d_ps = ps_d.tile([1, 512], F32, tag="d")
                    for i, (ks, pidx) in enumerate(row):
                        s_ps = ps_s.tile([128, 512], F32, tag="s")
                        nc.tensor.matmul(
                            s_ps[:], kT[ks // 4][:, (ks % 4) * 128:(ks % 4 + 1) * 128],
                            qT[hd][qt][:], start=True, stop=True)
                        w = wpool.tile([128, 512], BF16, tag="w")
                        nc.scalar.activation(w[:], s_ps[:], mybir.ActivationFunctionType.Exp)
                        if pidx is not None:
                            nc.vector.tensor_mul(
                                w[:], w[:], mk_sb[:, pidx * 512:(pidx + 1) * 512])
                        nc.tensor.matmul(a_ps[:], v_sb[ks // 4][:, (ks % 4) * 128:(ks % 4 + 1) * 128], w[:],
                                         start=(i == 0), stop=(i == len(row) - 1))
                        nc.tensor.matmul(d_ps[:], ones[:], w[:],
                                         start=(i == 0), stop=(i == len(row) - 1))
                    den = dpool.tile([1, 512], F32, tag="den")
                    nc.vector.tensor_scalar_add(den[:], d_ps[:], esk[0:1, hd:hd + 1])
                    rec = dpool.tile([1, 512], F32, tag="rec")
                    nc.vector.reciprocal_approx_fast(rec[:], den[:])
                    rbc = dpool.tile([128, 512], F32, tag="rbc")
                    nc.gpsimd.partition_broadcast(rbc[:], rec[:])
                    at = apool.tile([128, 512], F32R, tag="at")
                    nc.vector.tensor_mul(at[:], a_ps[:], rbc[:])
                    attnT.append(at)
                prev = (qt, attnT)
            for oc in range(HID // 512):
                oproj_chunk(prev[0], prev[1], oc)

    nc.compile()
    return nc


def kernel(hidden_states, positions, wq, wk, wv, wo, sink):
    global last_results
    h = np.asarray(hidden_states, np.float32)
    pos = np.asarray(positions)
    wq = np.asarray(wq, np.float32)
    wk = np.asarray(wk, np.float32)
    wv = np.asarray(wv, np.float32)
    wo = np.asarray(wo, np.float32)
    sink = np.asarray(sink, np.float32)

    masks, sched, n_pat = _schedule(pos)
    nc = _build(sched, n_pat)
    hT = np.ascontiguousarray(h.T)

    # RoPE tables (neox half-split), rows duplicated for both halves
    inv_freq = 1.0 / (THETA ** (np.arange(0, D, 2, dtype=np.float64) / D))
    freqs = pos.astype(np.float64)[:, None] * inv_freq[None, :]       # [S, 64]
    cos = np.cos(freqs).astype(np.float32).T                          # [64, S]
    sin = np.sin(freqs).astype(np.float32).T
    cos_full = np.concatenate([cos, cos], axis=0)                     # [128, S]
    sin_sign = np.concatenate([-sin, sin], axis=0)

    scale = np.float32(D ** -0.5)
    ones = np.ones((128, 1), np.float32)
    esink = np.exp(sink.astype(np.float64)).astype(np.float32)

    in_maps = []
    for c in range(CORES):
        in_maps.append({
            "ht": hT,
            "wq": np.ascontiguousarray(wq[:, c * DQ:(c + 1) * DQ] * scale),
            "wk": np.ascontiguousarray(wk[:, c * D:(c + 1) * D]),
            "wv": np.ascontiguousarray(wv[:, c * D:(c + 1) * D]),
            "wo": np.ascontiguousarray(wo[c * DQ:(c + 1) * DQ, :]),
            "cos": cos_full,
            "sin": sin_sign,
            "mk": masks,
            "one": ones.astype(ml_dtypes.bfloat16),
            "esk": np.ascontiguousarray(esink[None, c * QH:(c + 1) * QH]),
        })

    trace = bool(int(os.environ.get("KERNEL_TRACE", "0")))
    res = run_bass_kernel_spmd(nc, in_maps, core_ids=list(range(CORES)), trace=trace)
    last_results = res
    out = np.zeros((S, HID), np.float64)
    for r in res.results:
        out += r["out"].astype(np.float64)
    return out.astype(np.float32)

